# revision 44
# baseline (speedup 1.0000x reference)
"""AM-softmax + hard-negative-mining loss (partial-FC style) on 8 TRN2 cores.

Strategy (classification/tensor parallel over the queue dim Q):
  - Loss is invariant to a permutation of the Q columns; blended weight
    w = mask*q1 + (1-mask)*q0 equals q0 exactly where mask == 0 (~90%).
    Host permutes columns into a shared "U" block (one matmul feeds both
    loss terms) and an "M" block (q0 and blended-w variants). Per core:
    NU=7168 U columns + 2x NM=1024 M columns = 9216 columns in 9 spans
    of 1024. U overflow spills into M slots; by construction
    spill + |mask!=0| == 8*NM whenever |mask!=0| <= 8*NM, so the fast
    path always fits. Zero-padded columns contribute exp(0)=1 to row
    sums (subtracted on host) and cos=0 top-k candidates (neutral).
  - fp8 matmuls: p and queue columns are scaled by 64 and quantized to
    float8e4 on host. MatmulPerfMode.DoubleRow contracts K=256 per
    instruction at 0.5 cycles/row -- 4x the fp32r rate. The ~2e-3
    per-cos quantization noise averages out far below the 2e-2 loss
    tolerance; the host subtracts exp(S*gt_fp8) (recomputed from the
    same quantized operands) when applying the margin correction, so no
    quantization mismatch enters at the ground-truth column.
  - Only rows with label != -1 need logsumexp; only outlier rows need
    top-k. Host permutes batch rows so outlier rows occupy whole
    128-row chunks. PSUM can only be drained by ACT and DVE, so a
    greedy balancer splits pos chunks between fused ACT exp+accumulate
    and a DVE chain (Schraudolph fast-exp via one fused multiply-add
    into int16 whose bits read as bf16 ~= exp, then a 4x-mode
    accumulate); DVE max8 drains outlier chunks in cos space. The
    fast-exp +4% bias costs ~7e-4 relative on the loss, well inside
    the 2e-2 gate.
  - Cross-core reduction (logsumexp merge, top-k merge, margin
    adjustment, masked means) happens on host in float64.
"""
import sys

sys.path.insert(0, "/opt/trn_rl_repo")

import numpy as np

B = 1024
Q = 65536
D = 512
MARGIN = 0.4
SCALE = 32.0
HARD_NEG = 10
NCORES = 8

SP = 64.0                 # p quantization scale
SQ = 64.0                 # queue quantization scale
PSC = SP * SQ             # psum value = PSC * cos
PW = 1024                 # psum tile / span width
SW = 512                  # matmul slice width (one PSUM bank)
KG = 2                    # k-groups of 256 (DoubleRow contracts 256)
KS = 2                    # k-subtiles of 128 per group

NU = 7 * PW               # 7168 shared columns per core
NM = PW                   # 1024 masked columns per core (per variant)
NSP = 9                   # spans per core: 7 U + M0 + M1
CC = NU + 2 * NM          # 9216 columns streamed per core

QS = Q // NCORES          # generic-fallback shard size
NSP_G = QS // PW          # generic-fallback span count

TRACE = False
LAST = {}

_NC_CACHE = {}


K1 = 1.4426950408889634       # log2(e): i16 = trunc(K1*psum + K2)
K2 = 16256.49                 # 127*128 + .49; fastexp(0) == 1.0 exactly

# Schedule tuning knobs (measured per-1024-chunk engine costs in ns, plus
# emission-order / DMA-ring options). PSUM is only drainable by ACT and
# DVE; Pool's 0.42-efficiency software ops are not worth their coupling.
TUNE = {
    "c_act": 1200.0,          # ACT: exp + fused accumulate, psum read
    "c_dve_a": 1260.0,        # DVE: tensor_scalar fast-exp psum -> int16
    "c_dve_b": 250.0,         # DVE: 4x-mode accumulate of the bf16 bitcast
    "c_max8": 1195.0,         # DVE: max8 on psum
    "bc_order": "i9",         # within-span chunk emission order
    "qbufs": 4,               # q-span prefetch depth
    "rings": "sp",            # DMA issue rings: "sp" | "both"
    "q0_pieces": 2,           # span-0 piece-tile count
    "last_order": None,       # optional custom bc order for the last span
    "pt_split": 2,            # leading pT bc-slices DMA'd individually
    "warm_pe": 12,            # dummy matmuls to climb the PE p-state ramp
    "warm_act": True,         # preload the ACT exp table during DMA wait
    "flips": (),              # (si, bc) chunks whose greedy choice inverts
}

_BC_ORDERS = {
    "std": [0, 1, 2, 3, 4, 5, 6, 7],
    "posfirst": [2, 3, 4, 5, 6, 7, 0, 1],
    "inter": [2, 3, 0, 4, 5, 1, 6, 7],
    "i3": [0, 2, 3, 4, 5, 1, 6, 7],
    "i9": [2, 3, 4, 0, 5, 6, 7, 1],
}


def _build_fast(n_out_chunks, n_pos_chunks, mixed):
    """fp8 DoubleRow module. Batch chunks [0, n_out_chunks) are outlier
    (DVE max8 on psum), [B//128 - n_pos_chunks, B//128) are pos
    (exp+sum, split between fused-ACT and DVE fast-exp chunks by a
    greedy balancer); `mixed` chunks in the middle run both."""
    key = ("fast", n_out_chunks, n_pos_chunks, mixed,
           tuple(sorted(TUNE.items())))
    if key in _NC_CACHE:
        return _NC_CACHE[key]
    import concourse.mybir as mybir
    import concourse.tile as tile
    from concourse import bacc

    dt = mybir.dt
    f8 = dt.float8e4
    EXP = mybir.ActivationFunctionType.Exp
    DR = mybir.MatmulPerfMode.DoubleRow
    MUL = mybir.AluOpType.mult
    ADD = mybir.AluOpType.add
    BC = B // 128

    nc = bacc.Bacc(None)
    # span-slice DMAs read one contiguous run per partition
    pT = nc.dram_tensor("pT", [128, KG, KS, B], f8, kind="ExternalInput")
    qT = nc.dram_tensor("qT", [128, NSP, KG, KS, PW], f8,
                        kind="ExternalInput")
    n_sum = n_pos_chunks + mixed
    n_cand = n_out_chunks + mixed
    OW = n_sum * NSP + n_cand * NSP * 8
    oall = nc.dram_tensor("oall", [128, OW], dt.float32,
                          kind="ExternalOutput")

    acts = [bc >= BC - n_pos_chunks - mixed for bc in range(BC)]
    dves = [bc < n_out_chunks + mixed for bc in range(BC)]

    C_ACT, C_DVE_A = TUNE["c_act"], TUNE["c_dve_a"]
    C_DVE_B, C_MAX8 = TUNE["c_dve_b"], TUNE["c_max8"]
    bc_order = _BC_ORDERS[TUNE["bc_order"]][:BC] if BC == 8 else list(range(BC))

    # greedy engine assignment for pos chunks: minimize the max of the
    # two psum-drain engine timelines. A: fused ACT exp+accum.
    # D: DVE fast-exp to int16, then DVE 4x-mode accumulate (both write
    # the same per-span slot semantics as ACT).
    engine_of = {}
    tA = tD = 0.0
    for si in range(NSP):
        for bc in bc_order:
            if dves[bc]:
                tD += C_MAX8
            if acts[bc]:
                pick_a = max(tA + C_ACT, tD) <= max(tA, tD + C_DVE_A + C_DVE_B)
                if (si, bc) in TUNE["flips"]:
                    pick_a = not pick_a
                if pick_a:
                    engine_of[(si, bc)] = "A"
                    tA += C_ACT
                else:
                    engine_of[(si, bc)] = "D"
                    tD += C_DVE_A + C_DVE_B

    with tile.TileContext(nc) as tc:
        with (
            tc.tile_pool(name="const", bufs=1) as cpool,
            tc.tile_pool(name="qin", bufs=TUNE["qbufs"]) as qpool,
            tc.tile_pool(name="accum", bufs=1) as apool,
            tc.tile_pool(name="scr", bufs=3) as spool,
            tc.tile_pool(name="fexp", bufs=4) as fpool,
            tc.tile_pool(name="ps", bufs=4, space="PSUM") as ps,
        ):
            ring2 = nc.scalar if TUNE["rings"] == "both" else nc.sync
            pTr = cpool.tile([128, KG, KS, B], f8, tag="pTr")
            # warmup: tiny zero matmuls climb the PE p-state ramp and the
            # ACT exp-table load happens while the first DMAs land
            wz = cpool.tile([128, KG, KS, SW], f8, tag="wz")
            nc.gpsimd.memset(wz[:], 0.0)
            if TUNE["warm_act"]:
                wa = cpool.tile([128, 8], dt.float32, tag="wa")
                nc.vector.memset(wa[:], 0.0)
                wo = cpool.tile([128, 8], dt.bfloat16, tag="wo")
                nc.scalar.activation(wo[:], wa[:], EXP, scale=1.0)
            # startup: span 0 of q (longest pole) first, then p in two
            # pieces. Host ships pT with its bc dim already permuted to
            # bc_order, so both pieces are single contiguous runs.
            NP0 = TUNE["q0_pieces"]
            P0W = PW // NP0
            q0p = [qpool.tile([128, KG, KS, P0W], f8, tag=f"q0p{j}",
                              name=f"q0p{j}") for j in range(NP0)]
            nc.sync.dma_start(q0p[0][:], qT[:, 0, :, :, 0:P0W])
            nsplit = TUNE["pt_split"]
            for bc in bc_order[:nsplit]:
                nc.sync.dma_start(pTr[:, :, :, bc * 128:(bc + 1) * 128],
                                  pT[:, :, :, bc * 128:(bc + 1) * 128])
            for j in range(1, NP0):
                nc.sync.dma_start(q0p[j][:],
                                  qT[:, 0, :, :, j * P0W:(j + 1) * P0W])
            for bc in bc_order[nsplit:]:
                nc.sync.dma_start(pTr[:, :, :, bc * 128:(bc + 1) * 128],
                                  pT[:, :, :, bc * 128:(bc + 1) * 128])
            if TUNE["warm_pe"]:
                wps = ps.tile([128, SW], dt.float32, tag="ps", name="wps")
                for _ in range(TUNE["warm_pe"]):
                    nc.tensor.matmul(wps[:], wz[:, 0, :, 0:128], wz[:, 1],
                                     start=True, stop=True, perf_mode=DR)

            out_t = apool.tile([128, OW], dt.float32, tag="out", name="out")
            cbase = n_sum * (NSP - 1)
            pb = n_sum * (NSP - 1) + n_cand * (NSP - 1) * 8

            for si in range(NSP):
                if si == NSP - 1:
                    # spans 0..NSP-2 fully drained once their chunks land;
                    # ship that part of the output while span NSP-1 runs
                    nc.sync.dma_start(oall[:, 0:pb], out_t[:, 0:pb])
                if si == 0:
                    qt = None
                else:
                    qt = qpool.tile([128, KG, KS, PW], f8, tag="q",
                                    name=f"q{si}")
                    eng = nc.sync if si % 2 == 0 else ring2
                    eng.dma_start(qt[:], qT[:, si])
                span_order = bc_order
                if si == NSP - 1 and TUNE["last_order"]:
                    span_order = _BC_ORDERS[TUNE["last_order"]]
                for bc in span_order:
                    acc = ps.tile([128, PW], dt.float32, tag="ps",
                                  name=f"a{si}_{bc}")
                    if qt is None:
                        for j in range(NP0):
                            for h0 in range(0, P0W, SW):
                                hw = min(SW, P0W - h0)
                                for g in range(KG):
                                    nc.tensor.matmul(
                                        acc[:, j * P0W + h0:j * P0W + h0 + hw],
                                        pTr[:, g, :, bc * 128:(bc + 1) * 128],
                                        q0p[j][:, g, :, h0:h0 + hw],
                                        start=(g == 0),
                                        stop=(g == KG - 1),
                                        perf_mode=DR,
                                    )
                    else:
                        for h0 in range(0, PW, SW):
                            for g in range(KG):
                                nc.tensor.matmul(
                                    acc[:, h0:h0 + SW],
                                    pTr[:, g, :, bc * 128:(bc + 1) * 128],
                                    qt[:, g, :, h0:h0 + SW],
                                    start=(g == 0),
                                    stop=(g == KG - 1),
                                    perf_mode=DR,
                                )
                    if dves[bc]:
                        if si < NSP - 1:
                            c0 = cbase + bc * (NSP - 1) * 8 + si * 8
                        else:
                            c0 = pb + n_sum + bc * 8
                        nc.vector.max(
                            out=out_t[:, c0:c0 + 8], in_=acc[:, 0:PW])
                    if acts[bc]:
                        srow = bc - (BC - n_pos_chunks - mixed)
                        s0 = (srow * (NSP - 1) + si if si < NSP - 1
                              else pb + srow)
                        slot = out_t[:, s0:s0 + 1]
                        e = engine_of[(si, bc)]
                        if e == "A":
                            et = spool.tile([128, PW], dt.bfloat16, tag="et",
                                            name=f"e{si}_{bc}")
                            nc.scalar.activation(
                                et[:], acc[:, 0:PW], EXP, scale=SCALE / PSC,
                                accum_out=slot)
                            continue
                        it = fpool.tile([128, PW], dt.int16, tag="iF",
                                        name=f"i{si}_{bc}")
                        nc.vector.tensor_scalar(
                            out=it[:], in0=acc[:, 0:PW],
                            scalar1=K1, scalar2=K2, op0=MUL, op1=ADD)
                        bt = fpool.tile([128, PW], dt.bfloat16, tag="tF",
                                        name=f"b{si}_{bc}")
                        nc.vector.tensor_scalar(
                            out=bt[:], in0=it[:].bitcast(dt.bfloat16),
                            scalar1=1.0, scalar2=0.0, op0=MUL, op1=ADD,
                            accum_out=slot)

            nc.sync.dma_start(oall[:, pb:OW], out_t[:, pb:OW])

    nc.compile()
    _NC_CACHE[key] = nc
    return nc


def _emit_block_gen(nc, mybir, pools, pTr, src_dram, spans, sums_tiles,
                    cand_tiles, prefix):
    """fp32r fallback: matmul+exp+max8 over one column block."""
    dt = mybir.dt
    f32r = dt.float32r
    EXP = mybir.ActivationFunctionType.Exp
    DC = D // 128
    BC = B // 128
    qpool, spool, ps = pools
    off = 0
    for si, w in enumerate(spans):
        qt = qpool.tile([128, DC, PW], f32r, tag="q", name=f"{prefix}q{si}")
        for dc in range(DC):
            nc.sync.dma_start(
                qt[:, dc, 0:w], src_dram[:, dc, off:off + w].bitcast(f32r))
        for bc in range(BC):
            acc = ps.tile([128, PW], dt.float32, tag="ps",
                          name=f"{prefix}a{si}_{bc}")
            for h0 in range(0, w, SW):
                hw = min(SW, w - h0)
                for dc in range(DC):
                    nc.tensor.matmul(
                        acc[:, h0:h0 + hw],
                        pTr[:, dc, bc * 128:(bc + 1) * 128],
                        qt[:, dc, h0:h0 + hw],
                        start=(dc == 0),
                        stop=(dc == DC - 1),
                    )
            et = spool.tile([128, PW], dt.float32, tag="et",
                            name=f"{prefix}e{si}_{bc}")
            nc.scalar.activation(
                et[:, 0:w], acc[:, 0:w], EXP, scale=SCALE,
                accum_out=sums_tiles[bc][:, si:si + 1],
            )
            nc.vector.max(
                out=cand_tiles[bc][:, si * 8:(si + 1) * 8], in_=et[:, 0:w])
        off += w


def _build_generic():
    """Fallback: every column handled as masked (2 matmuls per column)."""
    if "gen" in _NC_CACHE:
        return _NC_CACHE["gen"]
    import concourse.mybir as mybir
    import concourse.tile as tile
    from concourse import bacc

    dt = mybir.dt
    DC = D // 128
    BC = B // 128
    nc = bacc.Bacc(None)
    pT = nc.dram_tensor("pT", [DC, 128, B], dt.float32, kind="ExternalInput")
    q0T = nc.dram_tensor("q0T", [128, DC, QS], dt.float32, kind="ExternalInput")
    wT = nc.dram_tensor("wT", [128, DC, QS], dt.float32, kind="ExternalInput")
    osums = nc.dram_tensor("osums", [2, BC, 128, NSP_G], dt.float32,
                           kind="ExternalOutput")
    ocand = nc.dram_tensor("ocand", [2, BC, 128, NSP_G * 8], dt.float32,
                           kind="ExternalOutput")

    with tile.TileContext(nc) as tc:
        with (
            tc.tile_pool(name="const", bufs=1) as cpool,
            tc.tile_pool(name="qin", bufs=TUNE["qbufs"]) as qpool,
            tc.tile_pool(name="accum", bufs=1) as apool,
            tc.tile_pool(name="scr", bufs=3) as spool,
            tc.tile_pool(name="ps", bufs=4, space="PSUM") as ps,
        ):
            f32r = dt.float32r
            pTr = cpool.tile([128, DC, B], f32r, tag="pTr")
            for dc in range(DC):
                nc.sync.dma_start(pTr[:, dc, :], pT[dc].bitcast(f32r))

            sums = [[apool.tile([128, NSP_G], dt.float32, tag=f"s{m}_{bc}",
                                name=f"s{m}_{bc}") for bc in range(BC)]
                    for m in range(2)]
            cand = [[apool.tile([128, NSP_G * 8], dt.float32, tag=f"c{m}_{bc}",
                                name=f"c{m}_{bc}") for bc in range(BC)]
                    for m in range(2)]

            pools = (qpool, spool, ps)
            spans = [PW] * NSP_G
            _emit_block_gen(nc, mybir, pools, pTr, q0T, spans, sums[0],
                            cand[0], "g0")
            _emit_block_gen(nc, mybir, pools, pTr, wT, spans, sums[1],
                            cand[1], "g1")

            for m in range(2):
                for bc in range(BC):
                    nc.sync.dma_start(osums[m, bc], sums[m][bc][:])
                    nc.sync.dma_start(ocand[m, bc], cand[m][bc][:])

    nc.compile()
    _NC_CACHE["gen"] = nc
    return nc


def _f8(x):
    import ml_dtypes
    return np.asarray(x, dtype=np.float32).astype(ml_dtypes.float8_e4m3)


def _layoutT8(cols_f8, n_cols):
    """[k, D] fp8 rows (k <= n_cols) -> [128, KG, KS, n_cols] fp8, zero
    padded; element (kk, g, i, j) = cols[j, g*256 + i*128 + kk]."""
    out = np.zeros((128, KG, KS, n_cols), dtype=cols_f8.dtype)
    k = cols_f8.shape[0]
    if k:
        out[:, :, :, :k] = cols_f8.reshape(k, KG, KS, 128).transpose(3, 1, 2, 0)
    return out


def _layoutT_gen(cols_2d, n_cols):
    DC = D // 128
    out = np.zeros((128, DC, n_cols), dtype=np.float32)
    k = cols_2d.shape[0]
    if k:
        t = np.ascontiguousarray(cols_2d.T).reshape(DC, 128, k)
        out[:, :, :k] = t.transpose(1, 0, 2)
    return np.ascontiguousarray(out)


def _run(nc, in_maps, core_ids):
    from concourse.bass_utils import run_bass_kernel_spmd
    kw = {}
    if TRACE:
        kw = dict(trace=True, trace_cores=[0])
    try:
        res = run_bass_kernel_spmd(nc, in_maps, core_ids, **kw)
    except ModuleNotFoundError:
        res = run_bass_kernel_spmd(nc, in_maps, core_ids)
    LAST["res"] = res
    return res


def kernel(p, queue, mask, label):
    p = np.ascontiguousarray(np.asarray(p, dtype=np.float32))
    queue = np.asarray(queue, dtype=np.float32)
    mask_flat = np.asarray(mask, dtype=np.float32).reshape(-1)
    label = np.asarray(label).astype(np.int64).reshape(-1)

    mask_nz = mask_flat != 0.0
    idx_M = np.nonzero(mask_nz)[0]
    idx_U = np.nonzero(~mask_nz)[0]
    if len(idx_M) > NCORES * NM:
        return _kernel_generic(p, queue, mask_flat, label)

    # --- column partition: U spill moves into M slots (they then carry
    # q0 / w==q0 variants, each loss still counts every column once) ---
    spill = max(0, len(idx_U) - NCORES * NU)
    if spill:
        idx_M = np.concatenate([idx_M, idx_U[-spill:]])
        idx_U = idx_U[:-spill]

    # --- batch-row permutation: outlier rows first, pos rows last ---
    pos_mask = label != -1
    n_pos = int(pos_mask.sum())
    n_neg = B - n_pos
    out_rows = np.nonzero(~pos_mask)[0]
    pos_rows = np.nonzero(pos_mask)[0]
    perm_rows = np.concatenate([out_rows, pos_rows])
    BC = B // 128
    n_out_chunks = n_neg // 128
    n_pos_chunks = n_pos // 128
    mixed = 1 if (n_out_chunks + n_pos_chunks) < BC else 0

    p_perm = p[perm_rows]
    p8 = _f8(p_perm * SP)
    pT8 = np.ascontiguousarray(
        p8.reshape(B, KG, KS, 128).transpose(3, 1, 2, 0))

    q0 = queue[0]
    mcolM = mask_flat[idx_M][:, None]
    wM = (mcolM * queue[1, idx_M, :]
          + (1.0 - mcolM) * queue[0, idx_M, :]).astype(np.float32)

    core_ids = list(range(NCORES))
    in_maps = []
    n_padU = n_padM = 0
    for c in core_ids:
        iu = idx_U[c * NU:(c + 1) * NU]
        sel = idx_M[c * NM:(c + 1) * NM]
        qu8 = _f8(q0[iu, :] * SQ)
        q0m8 = _f8(q0[sel, :] * SQ)
        w8 = _f8(wM[c * NM:(c + 1) * NM][:len(sel)] * SQ)
        qt = np.empty((128, NSP, KG, KS, PW), dtype=p8.dtype)
        lu = _layoutT8(qu8, NU).reshape(128, KG, KS, 7, PW)
        qt[:, 0:7] = lu.transpose(0, 3, 1, 2, 4)
        qt[:, 7] = _layoutT8(q0m8, NM)
        qt[:, 8] = _layoutT8(w8, NM)
        in_maps.append({"pT": pT8, "qT": np.ascontiguousarray(qt)})
        n_padU += NU - len(iu)
        n_padM += NM - len(sel)

    nc = _build_fast(n_out_chunks, n_pos_chunks, mixed)
    res = _run(nc, in_maps, core_ids)

    # ---- host-side reduction (float64) ----
    n_sum = n_pos_chunks + mixed
    n_cand = n_out_chunks + mixed
    sum_row0 = (BC - n_sum) * 128          # first permuted row with sums
    cand_row1 = n_cand * 128               # end of permuted rows with cands

    # per-loss z for the permuted rows that have sums
    z1 = np.zeros((B,), dtype=np.float64)
    z2 = np.zeros((B,), dtype=np.float64)
    cand_list = []
    for c in core_ids:
        r = res.results[c]
        oa = r["oall"].astype(np.float64)           # [128, OW]
        pb = n_sum * 8 + n_cand * 64
        su = np.empty((n_sum, 128, NSP))
        su[:, :, 0:8] = (oa[:, 0:n_sum * 8].T
                         .reshape(n_sum, 8, 128).transpose(0, 2, 1))
        su[:, :, 8] = oa[:, pb:pb + n_sum].T
        u_part = su[:, :, 0:7].sum(axis=2).reshape(-1)
        z1[sum_row0:] += u_part + su[:, :, 7].reshape(-1)
        z2[sum_row0:] += u_part + su[:, :, 8].reshape(-1)
        cd = np.empty((n_cand, 128, NSP * 8))
        cd[:, :, 0:64] = (oa[:, n_sum * 8:pb].T
                          .reshape(n_cand, 64, 128).transpose(0, 2, 1))
        cd[:, :, 64:72] = (oa[:, pb + n_sum:].T
                           .reshape(n_cand, 8, 128).transpose(0, 2, 1))
        cd /= PSC
        cand_list.append(cd.reshape(-1, NSP * 8))
    # zero pad columns contributed exp(0) = 1 each
    z1 -= n_padU + n_padM
    z2 -= n_padU + n_padM
    cand_all = np.concatenate(cand_list, axis=1)    # [cand_rows, 576]

    # candidate split per loss (per core: 9 spans * 8; 0..6 U, 7 M0, 8 M1)
    ncores_cols = cand_all.reshape(-1, NCORES, NSP, 8)
    c1 = ncores_cols[:, :, [0, 1, 2, 3, 4, 5, 6, 7], :].reshape(-1, NCORES * 64)
    c2 = ncores_cols[:, :, [0, 1, 2, 3, 4, 5, 6, 8], :].reshape(-1, NCORES * 64)

    # ---- ground-truth logits: exact for ce, fp8-matched for the z fix ----
    label_perm = label[perm_rows]
    q64 = queue.astype(np.float64)
    m64 = mask_flat.astype(np.float64)
    p64 = p_perm.astype(np.float64)
    p8f = p8.astype(np.float64)

    loss = 0.0
    pos_sel = np.arange(sum_row0, B)
    pos_valid = label_perm[pos_sel] != -1
    pos_rows_p = pos_sel[pos_valid]         # permuted-row ids with sums+label
    lbl = label_perm[pos_rows_p]
    if n_pos > 0:
        for m, z in ((0, z1), (1, z2)):
            if m == 0:
                w_rows = q64[0, lbl, :]
            else:
                mm = m64[lbl][:, None]
                w_rows = mm * q64[1, lbl, :] + (1.0 - mm) * q64[0, lbl, :]
            # fp8 column values exactly as the device saw them (f32 blend)
            w8_rows = _f8(w_rows.astype(np.float32) * SQ).astype(np.float64)
            gt = np.einsum("bd,bd->b", p64[pos_rows_p], w_rows)
            gt8 = np.einsum("bd,bd->b", p8f[pos_rows_p], w8_rows) / PSC
            zr = z[pos_rows_p]
            z_adj = (zr - np.exp(SCALE * gt8)
                     + np.exp(SCALE * (gt - MARGIN)))
            ce = np.log(z_adj) - (gt - MARGIN) * SCALE
            loss += ce.sum() / max(n_pos, 1)
    if n_neg > 0:
        neg_sel = np.arange(0, cand_row1)
        neg_valid = label_perm[neg_sel] == -1
        for cm in (c1, c2):
            cands_out = cm[neg_sel[neg_valid]]
            topk = -np.partition(-cands_out, HARD_NEG - 1,
                                 axis=1)[:, :HARD_NEG]
            hard = np.clip(topk, 0.0, None)
            loss += hard.mean(axis=1).sum() / max(n_neg, 1)

    return np.float32(loss)


def _kernel_generic(p, queue, mask_flat, label):
    mask_nz = mask_flat != 0.0
    idx_M = np.nonzero(mask_nz)[0]
    idx_U = np.nonzero(~mask_nz)[0]
    perm = np.concatenate([idx_U, idx_M])
    q0p = queue[0, perm, :]
    mcol = mask_flat[perm][:, None]
    wp = (mcol * queue[1, perm, :] + (1.0 - mcol) * queue[0, perm, :]
          ).astype(np.float32)
    DC = D // 128
    pT = np.ascontiguousarray(p.T).reshape(DC, 128, B)
    core_ids = list(range(NCORES))
    in_maps = []
    for c in core_ids:
        sl = slice(c * QS, (c + 1) * QS)
        in_maps.append({
            "pT": pT,
            "q0T": _layoutT_gen(q0p[sl], QS),
            "wT": _layoutT_gen(wp[sl], QS),
        })
    nc = _build_generic()
    res = _run(nc, in_maps, core_ids)

    sums_all = np.zeros((2, B), dtype=np.float64)
    cands = [[], []]
    for c in core_ids:
        r = res.results[c]
        sums_all += r["osums"].astype(np.float64).sum(axis=3).reshape(2, B)
        cm = r["ocand"].astype(np.float64).reshape(2, B, NSP_G * 8)
        cands[0].append(cm[0])
        cands[1].append(cm[1])
    with np.errstate(divide="ignore"):
        cand_all = [np.log(np.concatenate(cands[0], axis=1)) / SCALE,
                    np.log(np.concatenate(cands[1], axis=1)) / SCALE]

    pos_mask = label != -1
    n_pos = int(pos_mask.sum())
    n_neg = B - n_pos

    p64 = p.astype(np.float64)
    q64 = queue.astype(np.float64)
    m64 = mask_flat.astype(np.float64)

    loss = 0.0
    for m in range(2):
        if n_pos > 0:
            lbl = label[pos_mask]
            if m == 0:
                w_rows = q64[0, lbl, :]
            else:
                mm = m64[lbl][:, None]
                w_rows = mm * q64[1, lbl, :] + (1.0 - mm) * q64[0, lbl, :]
            gt = np.einsum("bd,bd->b", p64[pos_mask], w_rows)
            z = sums_all[m][pos_mask]
            z_adj = z - np.exp(SCALE * gt) + np.exp(SCALE * (gt - MARGIN))
            ce = np.log(z_adj) - (gt - MARGIN) * SCALE
            loss += ce.sum() / max(n_pos, 1)
        if n_neg > 0:
            cands_out = cand_all[m][~pos_mask]
            topk = -np.partition(-cands_out, HARD_NEG - 1,
                                 axis=1)[:, :HARD_NEG]
            hard = np.clip(topk, 0.0, None)
            loss += hard.mean(axis=1).sum() / max(n_neg, 1)

    return np.float32(loss)
